# revision 1
# baseline (speedup 1.0000x reference)
"""Trainium2 Bass kernel for the pairwise-MLP geometric convolution.

Reference computes, per batch z:
    rel[a,b]   = g[b] - g[a]
    h[a,b,:]   = relu(rel @ W1 + b1)                      [N,N,H]
    k[a,b,:]   = h @ W2 + b2  -> [N,N,C_OUT,C_IN]
    out[a,i]   = sum_{b,j} k[a,b,i,j] * f[b,j]

Key factorization (avoids materializing k, 537MB -> ~1MB):
    U = g @ W1  (so rel@W1 = U[b]-U[a])
    G[b,h,i]   = sum_j W2[h, i*C_IN+j] * f[b,j]
    out[a,i]   = sum_{b,h} relu(U[b,h]+b1[h]-U[a,h]) * G[b,h,i]
               + sum_j b2[i,j] * (sum_b f[b,j])

Sharding over 8 cores: z (2) x b-quarter (4). Each core computes the full
[i=32, a=256] transposed partial for its 64 b's; host sums quarters and
transposes. Contraction runs on the PE as 32 accumulating matmuls with
K=128 chunks of (b-pair x 64 h): stationary G chunk [128,32], moving
T chunk [128,256] built by one fused tensor_scalar (add bias, relu) per
chunk, spread across DVE/ACT/GPSIMD.

Hardware constraint honored throughout: a PE Matmult can carry at most ONE
sync-wait, so all small inputs arrive in a single packed DMA, and two
dummy matmuls make the PE observe the two g_sb gather DMAs before the
main accumulation chain.
"""

import os
import sys

import numpy as np

_TRN_REPO = "/opt/trn_rl_repo"
if _TRN_REPO not in sys.path:
    sys.path.insert(0, _TRN_REPO)

from contextlib import ExitStack

import concourse.bass as bass
import concourse.mybir as mybir
import concourse.tile as tile
from concourse.bass_utils import run_bass_kernel_spmd

from concourse.vector_clock import ScopedClock

# The walrus codegen used on the axon/PJRT path accepts at most ONE sync-wait
# per TPB instruction. Tile's kernel-tail drain aggregates a wait for every
# live semaphore onto a single Drain, which walrus rejects. Patch the tail to
# spread those waits across single-wait SP nops before an unadorned drain.
_orig_drain_and_barrier = tile.TileContext._drain_and_barrier


def _split_wait_drain_and_barrier(self, tick_clock, wait_clock):
    nc = self.nc
    probe = nc.sync.nop(nofuse=True)
    wait_clock.add_sem_waits(probe.ins, ScopedClock({None: tick_clock.global_clock}))
    si = probe.ins.sync_info
    waits = list(si.on_wait) if si is not None and si.on_wait else []
    if len(waits) > 1:
        probe.ins.sync_info = mybir.SyncInfo(on_wait=waits[:1], on_update=[])
        for w in waits[1:]:
            extra = nc.sync.nop(nofuse=True)
            extra.ins.sync_info = mybir.SyncInfo(on_wait=[w], on_update=[])
    nc.sync.drain()
    nc.all_engine_barrier()
    popped = nc._tile_sem_poison_stack.pop()
    assert popped is self._sem_poison
    nc.clear_and_free_semaphores(list(self.sems.allocated().values()))
    nc.all_engine_barrier()


tile.TileContext._drain_and_barrier = _split_wait_drain_and_barrier

F32 = mybir.dt.float32
# bf16 runs the PE at 1 cycle/row vs 4 for fp32; accumulation stays fp32 in
# PSUM. Only the big contraction operands (T, G) are bf16.
BF16 = mybir.dt.bfloat16
Z, N, C_IN, C_OUT, H = 2, 256, 32, 32, 64
BQ = 64          # b-points per core (N / 4 quarters)
NPAIR = BQ // 2  # 32 K-chunks of (2 b x 64 h) = 128

# packed small-input tensor layout (fp32): [64, PKW]
#   cols 0:256    fTfull   (parts 0:32)
#   cols 256:288  b2T      (parts 0:32)
#   col  288      b1c      (parts 0:64)
PKW = 289
# bf16 packed tensor (matmul operands), loaded as two DMAs (cols 0:MA,
# MA:MPW) so the U and first G' matmuls start before the whole tensor lands:
#   cols 0:64       fTq      (parts 0:32)
#   cols 64:320     gT       (parts 0:3)
#   cols 320:384    gTb      (parts 0:3)
#   cols 384:448    W1       (parts 0:3)
#   cols 448:2496   M2p
MPW = 2496
MA = 1472

# engine for each of the 32 T-chunk builds: v=vector(DVE), s=scalar(ACT),
# g=gpsimd. ACT carries the shared prep, DVE the G copies.
T_ENGINES = ["g", "s", "v", "g", "s", "g", "s", "v"] * 4
# PE warm-up matmuls between the G' matmuls and the main chain.
N_WARMERS = 16


def build_nc(debug: bool = False) -> bass.Bass:
    nc = bass.Bass("TRN2", target_bir_lowering=False, debug=debug, num_devices=8)

    m2p = nc.dram_tensor("M2p", [C_IN, MPW], BF16, kind="ExternalInput").ap()
    pk = nc.dram_tensor("pk", [H, PKW], F32, kind="ExternalInput").ap()
    outp = nc.dram_tensor("outp", [C_OUT, N], F32, kind="ExternalOutput").ap()

    with tile.TileContext(nc) as tc, ExitStack() as ctx:
        consts = ctx.enter_context(tc.tile_pool(name="consts", bufs=1))
        work = ctx.enter_context(tc.tile_pool(name="work", bufs=1))
        # bufs=NPAIR: every T tile gets its own slot, so no T-op ever waits
        # for a PE slot release (keeps every instruction at <=1 sync wait,
        # a walrus codegen hard limit).
        tpool = ctx.enter_context(tc.tile_pool(name="tpool", bufs=NPAIR))
        psum = ctx.enter_context(tc.tile_pool(name="psum", bufs=1, space="PSUM"))
        dpool = ctx.enter_context(tc.tile_pool(name="dpool", bufs=1, space="DRAM"))

        # ---- input loads. pk goes through the Pool SWDGE queue so the SP
        # HWDGE ring stays within 8 DMAs (no semaphore-lane reuse).
        m2p_sb = consts.tile([C_IN, MPW], BF16)
        nc.sync.dma_start(out=m2p_sb[:, 0:MA], in_=m2p[:, 0:MA])
        nc.sync.dma_start(out=m2p_sb[:, MA:MPW], in_=m2p[:, MA:MPW])
        pk_sb = consts.tile([H, PKW], F32)
        nc.gpsimd.dma_start(out=pk_sb, in_=pk)

        fTq_bf = m2p_sb[:, 0:64]
        gT_bf = m2p_sb[0:3, 64:320]
        gTb_bf = m2p_sb[0:3, 320:384]
        w1_bf = m2p_sb[0:3, 384:448]
        fTfull_sb = pk_sb[0:C_IN, 0:256]
        b2t_sb = pk_sb[0:C_IN, 256:288]
        b1_sb = pk_sb[0:H, 288:289]

        # First DVE / ACT ops must observe only the pk DMA semaphore.
        scol = work.tile([C_IN, 1], F32)
        nc.vector.tensor_reduce(out=scol, in_=fTfull_sb,
                                axis=mybir.AxisListType.X, op=mybir.AluOpType.add)
        s_bcast = work.tile([C_IN, N], BF16)
        nc.vector.tensor_scalar(out=s_bcast, in0=scol.broadcast_to([C_IN, N]),
                                scalar1=0.0, scalar2=None,
                                op0=mybir.AluOpType.add)
        b2t_bf = work.tile([C_IN, C_OUT], BF16)
        nc.vector.tensor_copy(b2t_bf, b2t_sb)

        # ---- U matmuls: U^T = W1^T @ g^T (bf16 in, fp32 accumulate).
        # Both U results share one PSUM bank, freeing a bank for the
        # warm-up matmuls.
        u_ps = psum.tile([H, N + BQ], F32)
        uaT_ps = u_ps[:, 0:N]
        ubT_ps = u_ps[:, N:N + BQ]
        nc.tensor.matmul(uaT_ps, lhsT=w1_bf, rhs=gT_bf, start=True, stop=True)
        nc.tensor.matmul(ubT_ps, lhsT=w1_bf, rhs=gTb_bf, start=True, stop=True)

        # All shared T-op inputs are produced on ACT so T consumers on any
        # engine need exactly one (ACT) wait. negUa duplicated on both
        # partition halves: [128, N].
        negua2 = work.tile([2 * H, N], F32)
        nc.scalar.activation(negua2[0:H, :], uaT_ps,
                             mybir.ActivationFunctionType.Copy, scale=-1.0)
        nc.scalar.activation(negua2[H:2 * H, :], uaT_ps,
                             mybir.ActivationFunctionType.Copy, scale=-1.0)

        # Ub + b1, then stacked by pair: ubT2[bl*H+h, p] = Ub[2p+bl, h] + b1[h]
        ubB = work.tile([H, BQ], F32)
        nc.vector.tensor_scalar(out=ubB, in0=ubT_ps, scalar1=b1_sb,
                                scalar2=None, op0=mybir.AluOpType.add)
        ubT2 = work.tile([2 * H, NPAIR], F32)
        ubB_r = ubB.rearrange("h (p two) -> h two p", two=2)
        nc.scalar.activation(ubT2[0:H, :], ubB_r[:, 0, :],
                             mybir.ActivationFunctionType.Copy)
        nc.scalar.activation(ubT2[H:2 * H, :], ubB_r[:, 1, :],
                             mybir.ActivationFunctionType.Copy)

        # ---- G: G'[b, h*32+i] = sum_j fTq[j,b] * M2p[j, h*32+i] ----
        g_ps = []
        for k in range(4):
            gp = psum.tile([BQ, 512], F32, name=f"g_ps{k}", tag=f"g_ps{k}")
            nc.tensor.matmul(gp, lhsT=fTq_bf,
                             rhs=m2p_sb[:, 448 + k * 512:448 + (k + 1) * 512],
                             start=True, stop=True)
            g_ps.append(gp)

        # PSUM -> SBUF on DVE (DMA cannot read PSUM), then bounce through
        # DRAM to regroup (b-pair, h) onto partitions.
        g_tmp = work.tile([BQ, H * C_OUT], BF16)
        for k in range(4):
            nc.vector.tensor_copy(g_tmp[:, k * 512:(k + 1) * 512], g_ps[k])
        g_sb = work.tile([2 * H, NPAIR, C_OUT], BF16)
        g_dram = dpool.tile([BQ, H * C_OUT], BF16)
        nc.sync.dma_start(out=g_dram, in_=g_tmp)
        # Two gathers split by p-half. Because 64 h * 32 i = 2048 = the
        # g_dram row stride, the (bl, h) pair merges into ONE uniform
        # stride-32 dim, keeping each side a legal 3D AP:
        #   src element (2p+bl, h*32+i) -> offset (bl*64+h)*32 + p*4096 + i
        g0 = g_dram[:, :]
        for ph in range(2):
            g_src = bass.AP(tensor=g0.tensor,
                            offset=g0.offset + ph * 16 * 4096,
                            ap=[[32, 2 * H], [4096, 16], [1, C_OUT]])
            nc.sync.dma_start(out=g_sb[:, 16 * ph:16 * (ph + 1), :],
                              in_=g_src)

        # ---- b2 bias term first in the acc group ----
        acc = psum.tile([C_OUT, N], F32)
        nc.tensor.matmul(acc, lhsT=b2t_bf, rhs=s_bcast, start=True, stop=False)

        scrap = psum.tile([C_OUT, 1], F32)

        def observe_gather(ph):
            # PE observes the p-half gather (one wait) so the following
            # main matmuls need only their T-tile wait.
            nc.tensor.matmul(scrap, lhsT=g_sb[:, 16 * ph, :],
                             rhs=g_sb[:, 16 * ph, 0:1],
                             start=True, stop=True)

        # ---- main contraction: acc[i, a] += G_p^T @ T_p ----
        # T-gated PE warm-up: warmer w consumes t_w as it is produced, so
        # the PE tracks T production (staying at high p-state) instead of
        # idling while the G gathers are in flight.
        warm_ps = psum.tile([C_OUT, N], F32)
        t_tiles = []
        for p in range(NPAIR):
            t_p = tpool.tile([2 * H, N], BF16, tag="T", name=f"t_{p}")
            t_tiles.append(t_p)
            eng = T_ENGINES[p]
            if eng == "s":
                nc.scalar.activation(t_p, negua2,
                                     mybir.ActivationFunctionType.Relu,
                                     bias=ubT2[:, p:p + 1], scale=1.0)
            else:
                e = nc.vector if eng == "v" else nc.gpsimd
                e.tensor_scalar(out=t_p, in0=negua2,
                                scalar1=ubT2[:, p:p + 1], scalar2=0.0,
                                op0=mybir.AluOpType.add,
                                op1=mybir.AluOpType.max)
            if p < N_WARMERS:
                nc.tensor.matmul(warm_ps, lhsT=t_p[0:C_IN, 0:C_OUT],
                                 rhs=t_p[0:C_IN, :], start=True, stop=True)
        for ph in range(2):
            observe_gather(ph)
            for p in range(16 * ph, 16 * (ph + 1)):
                nc.tensor.matmul(acc, lhsT=g_sb[:, p, :], rhs=t_tiles[p],
                                 start=False, stop=(p == NPAIR - 1))

        # ---- store ----
        out_sb = work.tile([C_OUT, N], F32)
        nc.scalar.activation(out_sb, acc, mybir.ActivationFunctionType.Copy)
        nc.sync.dma_start(out=outp, in_=out_sb)

    return nc


def shard_inputs(features, geometry, W1, b1, W2, b2) -> list[dict]:
    import ml_dtypes
    bf16 = ml_dtypes.bfloat16
    f = np.ascontiguousarray(np.asarray(features, np.float32))
    g = np.ascontiguousarray(np.asarray(geometry, np.float32))
    W1 = np.ascontiguousarray(np.asarray(W1, np.float32))
    b1 = np.ascontiguousarray(np.asarray(b1, np.float32))
    W2 = np.ascontiguousarray(np.asarray(W2, np.float32))
    b2 = np.ascontiguousarray(np.asarray(b2, np.float32))

    m2p = W2.reshape(H, C_OUT, C_IN).transpose(2, 0, 1).reshape(C_IN, H * C_OUT)
    b2t = np.ascontiguousarray(b2.reshape(C_OUT, C_IN).T)

    maps = []
    for core in range(8):
        z, q = divmod(core, 4)
        sl = slice(q * BQ, (q + 1) * BQ)
        pk = np.zeros((H, PKW), np.float32)
        pk[0:C_IN, 0:256] = f[z].T
        if q == 0:
            pk[0:C_IN, 256:288] = b2t
        pk[0:H, 288] = b1
        mp = np.zeros((C_IN, MPW), bf16)
        mp[:, 0:64] = f[z, sl].T.astype(bf16)
        mp[0:3, 64:320] = g[z].T.astype(bf16)
        mp[0:3, 320:384] = g[z, sl].T.astype(bf16)
        mp[0:3, 384:448] = W1.astype(bf16)
        mp[:, 448:2496] = m2p.astype(bf16)
        maps.append({"pk": pk, "M2p": mp})
    return maps


def unshard(parts: list[np.ndarray]) -> np.ndarray:
    out = np.empty((Z, N, C_OUT), np.float32)
    for z in range(Z):
        acc = parts[4 * z].astype(np.float32)
        for q in range(1, 4):
            acc = acc + parts[4 * z + q]
        out[z] = acc.T
    return out


def kernel(**inputs) -> np.ndarray:
    nc = build_nc(debug=False)
    in_maps = shard_inputs(**inputs)
    res = run_bass_kernel_spmd(nc, in_maps, list(range(8)))
    return unshard([r["outp"] for r in res.results])



# revision 5
# speedup vs baseline: 1.6040x; 1.6040x over previous
"""Trainium2 Bass kernel v2 for the pairwise-MLP geometric convolution.

Reference computes, per batch z:
    rel[a,b]   = g[b] - g[a]
    h[a,b,:]   = relu(rel @ W1 + b1)                      [N,N,H]
    k[a,b,:]   = h @ W2 + b2  -> [N,N,C_OUT,C_IN]
    out[a,i]   = sum_{b,j} k[a,b,i,j] * f[b,j]

Factorization (per core: one z, one b-quarter Q of 64 points):
    U = g @ W1
    G[b,h,i]   = sum_j W2[h, i*C_IN+j] * f[b,j]
    out[a,i]   = sum_{b in Q,h} relu(U[b,h]+b1[h]-U[a,h]) * G[b,h,i]
               + bias[i]                      (bias = b2 @ fsum, on host)

Design notes (driven by the TimelineSim cost model):
  * matmul cost = out-free-size x cycle (contraction K is free), so G is
    produced DIRECTLY in the pair layout [(bl,h), (p,i)] by many tiny
    matmuls (lhsT = W2 repacked [j, i*64+h], rhs = f^T quarter slices),
    killing v1's DRAM regroup bounce (~8us of critical path).
  * pairs are p = {q0+bl*32+p}: contiguous slices everywhere; b1 is
    folded into the U_b matmul via an ones row (K=4).
  * each DMA has ~2.2us fixed latency -> exactly 2 input DMAs and 2
    output DMAs (acc pair-halves, summed on host with the bias).
  * PE p-state ramps to full speed only after 3us of continuous busy.
    Warm-up matmuls are emitted LAST so the tile scheduler (which uses
    emission order as priority) treats them as gap fillers: PE never
    idles, and the main chain runs at 0.417 ns/row.
  * tile semaphores count per-slot, so independently consumed data gets
    its own tile (g2A/g2B psum by pair-half, accA/accB, g_sbufA/B).
"""

import os
import sys

import numpy as np

_TRN_REPO = "/opt/trn_rl_repo"
if _TRN_REPO not in sys.path:
    sys.path.insert(0, _TRN_REPO)

from contextlib import ExitStack

import concourse.bass as bass
import concourse.mybir as mybir
import concourse.tile as tile
from concourse.bass_utils import run_bass_kernel_spmd

from concourse.vector_clock import ScopedClock

# The walrus codegen used on the axon/PJRT path accepts at most ONE sync-wait
# per TPB instruction. Tile's kernel-tail drain aggregates a wait for every
# live semaphore onto a single Drain, which walrus rejects. Patch the tail to
# spread those waits across single-wait SP nops before an unadorned drain.
_orig_drain_and_barrier = tile.TileContext._drain_and_barrier


def _split_wait_drain_and_barrier(self, tick_clock, wait_clock):
    nc = self.nc
    probe = nc.sync.nop(nofuse=True)
    wait_clock.add_sem_waits(probe.ins, ScopedClock({None: tick_clock.global_clock}))
    si = probe.ins.sync_info
    waits = list(si.on_wait) if si is not None and si.on_wait else []
    if len(waits) > 1:
        probe.ins.sync_info = mybir.SyncInfo(on_wait=waits[:1], on_update=[])
        for w in waits[1:]:
            extra = nc.sync.nop(nofuse=True)
            extra.ins.sync_info = mybir.SyncInfo(on_wait=[w], on_update=[])
    nc.sync.drain()
    nc.all_engine_barrier()
    popped = nc._tile_sem_poison_stack.pop()
    assert popped is self._sem_poison
    nc.clear_and_free_semaphores(list(self.sems.allocated().values()))
    nc.all_engine_barrier()


tile.TileContext._drain_and_barrier = _split_wait_drain_and_barrier

F32 = mybir.dt.float32
BF16 = mybir.dt.bfloat16
Z, N, C_IN, C_OUT, H = 2, 256, 32, 32, 64
BQ = 64   # b-points per core (N / 4 quarters)
NP = 32   # pairs per core: pair p = {q0 + bl*32 + p : bl in 0,1}

# mpA (bf16 [32, MAW]): fTq 0:64 | M2v2 64:2112
MAW = 2112
# mpB (bf16 [4, MBW]): gT1 0:256 (row3=ones) | gTb1 256:320 (row3=ones)
#                      | W1b 320:384 (row3=b1) | W1neg 384:448 (row3=0)
MBW = 448

N_WARM = 48  # PE gap-filler matmuls, emitted last (lowest priority)
# engine per T tile: v=DVE (127ns), s=Act (398ns), g=Pool (451ns).
# The first 8 pairs alternate Act/Pool (DVE is reserved for the
# chain-gating copy-A); DVE then carries the bulk.
T_ENGINES = list("sgsgsgsg" + "vvvvvvvvvvvvvvvvvvvvvvvv")
for _k in (15, 19, 23, 27):
    T_ENGINES[_k] = "g"
assert len(T_ENGINES) == NP


def build_nc(debug: bool = False) -> bass.Bass:
    nc = bass.Bass("TRN2", target_bir_lowering=False, debug=debug, num_devices=8)

    mpA = nc.dram_tensor("mpA", [C_IN, MAW], BF16, kind="ExternalInput").ap()
    mpB = nc.dram_tensor("mpB", [4, MBW], BF16, kind="ExternalInput").ap()
    outp = nc.dram_tensor("outp", [C_OUT, 2 * N], F32, kind="ExternalOutput").ap()

    with tile.TileContext(nc) as tc, ExitStack() as ctx:
        consts = ctx.enter_context(tc.tile_pool(name="consts", bufs=1))
        work = ctx.enter_context(tc.tile_pool(name="work", bufs=1))
        tpool = ctx.enter_context(tc.tile_pool(name="tpool", bufs=NP))
        psum = ctx.enter_context(tc.tile_pool(name="psum", bufs=1, space="PSUM"))

        # ---- input loads, both on the SP HWDGE queue. B (the U path)
        # first: the whole T pipeline hangs off it, while the G path
        # tolerates mpA's later semaphore (~4.3us) ----
        mpB_sb = consts.tile([4, MBW], BF16, tag="mpB")
        nc.sync.dma_start(out=mpB_sb, in_=mpB)
        mpA_sb = consts.tile([C_IN, MAW], BF16, tag="mpA")
        nc.sync.dma_start(out=mpA_sb, in_=mpA)

        fTq = mpA_sb[:, 0:64]
        gT1 = mpB_sb[:, 0:256]
        gTb1 = mpB_sb[:, 256:320]
        w1b = mpB_sb[:, 320:384]
        w1neg = mpB_sb[:, 384:448]

        # ---- PSUM tiles (8 banks of 2KB/partition) ----
        uneg_ps = psum.tile([2 * H, N], F32)    # -U^T on both partition halves
        ubT_ps = psum.tile([H, BQ], F32)
        warm_ps = psum.tile([C_OUT, 128], F32)  # warmers + observers target
        g2A = psum.tile([2 * H, 512], F32)      # G psum, pairs 0:16  (i, p')
        g2B = psum.tile([2 * H, 512], F32)      # G psum, pairs 16:32
        accA = psum.tile([C_OUT, N], F32)
        accB = psum.tile([C_OUT, N], F32)

        # ---- U matmuls: uneg = (-W1)^T @ g^T twice (both partition halves);
        # ubT = W1b^T @ gTb1 (b1 folded in via the ones row) ----
        nc.tensor.matmul(uneg_ps[0:H, :], lhsT=w1neg[0:3, :], rhs=gT1[0:3, :],
                         start=True, stop=True)
        nc.tensor.matmul(uneg_ps[H:2 * H, :], lhsT=w1neg[0:3, :],
                         rhs=gT1[0:3, :], start=True, stop=True)
        nc.tensor.matmul(ubT_ps, lhsT=w1b, rhs=gTb1, start=True, stop=True)

        # ---- shared T inputs ----
        # negua2[(bl,h), a] = -U[a, h] (bf16 so T ops hit fast DVE modes)
        negua2 = work.tile([2 * H, N], BF16, tag="negua2")
        nc.vector.tensor_copy(negua2, uneg_ps)
        # ubB2[(bl,h), p] = U[q0+bl*32+p, h] + b1[h]  (psum -> sbuf copies;
        # on DVE — GPSIMD cannot read PSUM)
        ubB2 = work.tile([2 * H, NP], F32, tag="ubB2")
        nc.vector.tensor_copy(ubB2[0:H, :], ubT_ps[:, 0:32])
        nc.vector.tensor_copy(ubB2[H:2 * H, :], ubT_ps[:, 32:64])

        # ---- G matmuls, pair-half major so copies/chain overlap phase 2:
        # g2X[bl*64+h, i*16+p'] = G[q0+bl*32+(ph*16+p'), h, i] ----
        for ph, g2X in ((0, g2A), (1, g2B)):
            for bl in range(2):
                rhs = fTq[:, bl * 32 + ph * 16: bl * 32 + ph * 16 + 16]
                for i in range(C_OUT):
                    nc.tensor.matmul(
                        g2X[bl * H:(bl + 1) * H, i * 16:(i + 1) * 16],
                        lhsT=mpA_sb[:, 64 + i * 64:64 + (i + 1) * 64],
                        rhs=rhs, start=True, stop=True)

        # ---- G psum -> sbuf regroup: plain contiguous [128, 512] copies
        # (walrus rejects transposed copy APs); the (i, p') -> per-pair
        # transpose happens in the chain's lhsT access pattern instead.
        # g_sbuf[x][(bl,h), i*16 + p'] = g2X[(bl,h), i*16 + p']
        g_sbuf = [work.tile([2 * H, 512], BF16, name=f"gsb{x}",
                            tag=f"gsb{x}")
                  for x in range(2)]

        def g_copy(engine, x):
            g2X = (g2A, g2B)[x]
            if engine is nc.scalar:
                engine.activation(g_sbuf[x], g2X,
                                  mybir.ActivationFunctionType.Copy)
            else:
                engine.tensor_copy(g_sbuf[x], g2X)

        # lhsT view: [:, p, :] = [(bl,h), i] for pair p (free stride 16)
        g_lhs = [t.rearrange("k (i p) -> k p i", i=C_OUT) for t in g_sbuf]

        # copy-A gates the chain start: high priority on DVE (it pops the
        # moment G-A's semaphore lands). copy-B (Act) is emitted mid
        # T stream — it is only needed at pair 16.
        with tc.high_priority():
            g_copy(nc.vector, 0)

        # ---- T tiles ----
        t_tiles = [None] * NP

        def emit_t(p):
            t_p = tpool.tile([2 * H, N], BF16, tag="T", name=f"t_{p}")
            t_tiles[p] = t_p
            eng = T_ENGINES[p]
            if eng == "s":
                nc.scalar.activation(t_p, negua2,
                                     mybir.ActivationFunctionType.Relu,
                                     bias=ubB2[:, p:p + 1], scale=1.0)
            else:
                e = nc.vector if eng == "v" else nc.gpsimd
                with tc.tile_wait_until(5.2e-3, enable=(eng == "v")):
                    # Floor DVE T builds past copy-A's readiness in the
                    # scheduler's internal clock (which misses the fast DVE
                    # modes and would otherwise pack DVE before copy-A).
                    e.tensor_scalar(out=t_p, in0=negua2,
                                    scalar1=ubB2[:, p:p + 1], scalar2=0.0,
                                    op0=mybir.AluOpType.add,
                                    op1=mybir.AluOpType.max)

        for p in range(7):
            emit_t(p)
        g_copy(nc.scalar, 1)
        for p in range(7, NP):
            emit_t(p)

        # ---- observers: PE absorbs the g_sbuf copy semaphores so chain
        # matmuls carry only their T-tile wait (walrus: 1 wait/Matmult) ----
        def observe(x):
            nc.tensor.matmul(warm_ps[0:1, 0:1],
                             lhsT=g_sbuf[x][0:64, 0:1],
                             rhs=g_sbuf[x][0:64, 0:1],
                             start=True, stop=True)

        # ---- main chain: two pair-halves of acc, host sums them ----
        for p in range(NP):
            if p % 16 == 0:
                observe(p // 16)
            accX = accA if p < 16 else accB
            nc.tensor.matmul(accX, lhsT=g_lhs[p // 16][:, p % 16, :],
                             rhs=t_tiles[p],
                             start=(p % 16 == 0), stop=(p % 16 == 15))

        # ---- store: half A as soon as its group stops, half B at the end.
        # Both copies on Act (idle in this window) so DVE's T stream is
        # never interrupted ----
        out_sbA = work.tile([C_OUT, N], F32, tag="outA")
        out_sbB = work.tile([C_OUT, N], F32, tag="outB")
        with tc.high_priority():
            nc.scalar.activation(out_sbA, accA,
                                 mybir.ActivationFunctionType.Copy)
        nc.sync.dma_start(out=outp[:, 0:N], in_=out_sbA)
        with tc.high_priority():
            nc.scalar.activation(out_sbB, accB,
                                 mybir.ActivationFunctionType.Copy)
        nc.sync.dma_start(out=outp[:, N:2 * N], in_=out_sbB)

        # ---- PE warm-up / gap fillers: emitted last => lowest scheduler
        # priority => they run only when nothing else is ready on PE,
        # keeping the p-state ramp alive with zero semaphore waits ----
        wt_t = nc.alloc_sbuf_tensor("warm_src", [128, 128], BF16)
        wt = wt_t.ap()
        for _ in range(N_WARM):
            nc.tensor.matmul(warm_ps, lhsT=wt[:, 0:C_OUT], rhs=wt,
                             start=True, stop=True)

    # The framework registers four const-AP tensors ([128,1] fills) at Bass
    # init; nothing in this kernel reads them (the BIR verifier itself flags
    # them as reader-less), but their Pool memsets delay the preamble
    # all-engine barrier by ~450ns. Drop them.
    b0 = nc.m.functions[0].blocks[0]
    b0.instructions = [i for i in b0.instructions
                       if type(i).__name__ != "InstMemset"]
    return nc


def shard_inputs(features, geometry, W1, b1, W2, b2) -> list[dict]:
    import ml_dtypes
    bf16 = ml_dtypes.bfloat16
    f = np.ascontiguousarray(np.asarray(features, np.float32))
    g = np.ascontiguousarray(np.asarray(geometry, np.float32))
    W1 = np.ascontiguousarray(np.asarray(W1, np.float32))
    b1 = np.ascontiguousarray(np.asarray(b1, np.float32))
    W2 = np.ascontiguousarray(np.asarray(W2, np.float32))

    # M2v2[j, i*64+h] = W2[h, i*C_IN+j]
    m2v2 = W2.reshape(H, C_OUT, C_IN).transpose(2, 1, 0).reshape(C_IN, C_OUT * H)

    maps = []
    for core in range(8):
        z, q = divmod(core, 4)
        sl = slice(q * BQ, (q + 1) * BQ)
        mpA = np.zeros((C_IN, MAW), bf16)
        mpA[:, 0:64] = f[z, sl].T.astype(bf16)
        mpA[:, 64:2112] = m2v2.astype(bf16)
        mpB = np.zeros((4, MBW), bf16)
        mpB[0:3, 0:256] = g[z].T.astype(bf16)
        mpB[3, 0:256] = 1.0
        mpB[0:3, 256:320] = g[z, sl].T.astype(bf16)
        mpB[3, 256:320] = 1.0
        mpB[0:3, 320:384] = W1.astype(bf16)
        mpB[3, 320:384] = b1.astype(bf16)
        mpB[0:3, 384:448] = (-W1).astype(bf16)
        maps.append({"mpA": mpA, "mpB": mpB})
    return maps


def unshard(parts: list[np.ndarray], features, b2) -> np.ndarray:
    f = np.asarray(features, np.float32)
    b2m = np.asarray(b2, np.float32).reshape(C_OUT, C_IN)
    # bias[z, i] = sum_j b2[i, j] * (sum_b f[z, b, j]) — constant over a
    bias = f.sum(axis=1) @ b2m.T  # [Z, C_OUT]
    out = np.empty((Z, N, C_OUT), np.float32)
    for z in range(Z):
        acc = parts[4 * z][:, 0:N] + parts[4 * z][:, N:2 * N]
        for q in range(1, 4):
            acc = acc + parts[4 * z + q][:, 0:N] + parts[4 * z + q][:, N:2 * N]
        out[z] = acc.T + bias[z][None, :]
    return out


def kernel(**inputs) -> np.ndarray:
    nc = build_nc(debug=False)
    in_maps = shard_inputs(**inputs)
    res = run_bass_kernel_spmd(nc, in_maps, list(range(8)))
    return unshard([r["outp"] for r in res.results],
                   inputs["features"], inputs["b2"])


# revision 7
# speedup vs baseline: 1.6266x; 1.0141x over previous
"""Trainium2 Bass kernel v2 for the pairwise-MLP geometric convolution.

Reference computes, per batch z:
    rel[a,b]   = g[b] - g[a]
    h[a,b,:]   = relu(rel @ W1 + b1)                      [N,N,H]
    k[a,b,:]   = h @ W2 + b2  -> [N,N,C_OUT,C_IN]
    out[a,i]   = sum_{b,j} k[a,b,i,j] * f[b,j]

Factorization (per core: one z, one b-quarter Q of 64 points):
    U = g @ W1
    G[b,h,i]   = sum_j W2[h, i*C_IN+j] * f[b,j]
    out[a,i]   = sum_{b in Q,h} relu(U[b,h]+b1[h]-U[a,h]) * G[b,h,i]
               + bias[i]                      (bias = b2 @ fsum, on host)

Design notes (driven by the TimelineSim cost model):
  * matmul cost = out-free-size x cycle (contraction K is free), so G is
    produced DIRECTLY in the pair layout [(bl,h), (p,i)] by many tiny
    matmuls (lhsT = W2 repacked [j, i*64+h], rhs = f^T quarter slices),
    killing v1's DRAM regroup bounce (~8us of critical path).
  * pairs are p = {q0+bl*32+p}: contiguous slices everywhere; b1 is
    folded into the U_b matmul via an ones row (K=4).
  * each DMA has ~2.2us fixed latency -> exactly 2 input DMAs and 2
    output DMAs (acc pair-halves, summed on host with the bias).
  * PE p-state ramps to full speed only after 3us of continuous busy.
    Warm-up matmuls are emitted LAST so the tile scheduler (which uses
    emission order as priority) treats them as gap fillers: PE never
    idles, and the main chain runs at 0.417 ns/row.
  * tile semaphores count per-slot, so independently consumed data gets
    its own tile (g2A/g2B psum by pair-half, accA/accB, g_sbufA/B).
"""

import os
import sys

import numpy as np

_TRN_REPO = "/opt/trn_rl_repo"
if _TRN_REPO not in sys.path:
    sys.path.insert(0, _TRN_REPO)

from contextlib import ExitStack

import concourse.bass as bass
import concourse.mybir as mybir
import concourse.tile as tile
from concourse.bass_utils import run_bass_kernel_spmd

from concourse.vector_clock import ScopedClock

# The walrus codegen used on the axon/PJRT path accepts at most ONE sync-wait
# per TPB instruction. Tile's kernel-tail drain aggregates a wait for every
# live semaphore onto a single Drain, which walrus rejects. Patch the tail to
# spread those waits across single-wait SP nops before an unadorned drain.
_orig_drain_and_barrier = tile.TileContext._drain_and_barrier


def _split_wait_drain_and_barrier(self, tick_clock, wait_clock):
    nc = self.nc
    probe = nc.sync.nop(nofuse=True)
    wait_clock.add_sem_waits(probe.ins, ScopedClock({None: tick_clock.global_clock}))
    si = probe.ins.sync_info
    waits = list(si.on_wait) if si is not None and si.on_wait else []
    if len(waits) > 1:
        probe.ins.sync_info = mybir.SyncInfo(on_wait=waits[:1], on_update=[])
        for w in waits[1:]:
            extra = nc.sync.nop(nofuse=True)
            extra.ins.sync_info = mybir.SyncInfo(on_wait=[w], on_update=[])
    nc.sync.drain()
    nc.all_engine_barrier()
    popped = nc._tile_sem_poison_stack.pop()
    assert popped is self._sem_poison
    nc.clear_and_free_semaphores(list(self.sems.allocated().values()))
    nc.all_engine_barrier()


tile.TileContext._drain_and_barrier = _split_wait_drain_and_barrier

F32 = mybir.dt.float32
BF16 = mybir.dt.bfloat16
Z, N, C_IN, C_OUT, H = 2, 256, 32, 32, 64
BQ = 64   # b-points per core (N / 4 quarters)
NP = 32   # pairs per core: pair p = {q0 + bl*32 + p : bl in 0,1}

# mpA (bf16 [32, MAW]): fTq 0:64 | M2v2 64:2112
MAW = 2112
# mpB (bf16 [4, MBW]): gT1 0:256 (row3=ones) | gTb1 256:320 (row3=ones)
#                      | W1b 320:384 (row3=b1) | W1neg 384:448 (row3=0)
MBW = 448

N_WARM = 48  # PE gap-filler matmuls, emitted last (lowest priority)
# engine per T tile: v=DVE (127ns), s=Act (398ns), g=Pool (451ns).
# The first 8 pairs alternate Act/Pool (DVE is reserved for the
# chain-gating copy-A); DVE then carries the bulk.
T_ENGINES = ["v"] * NP
for _k in (0, 2, 24, 28):
    T_ENGINES[_k] = "s"
for _k in (1, 3, 7, 11, 15, 19, 23):
    T_ENGINES[_k] = "g"
assert len(T_ENGINES) == NP


def build_nc(debug: bool = False) -> bass.Bass:
    nc = bass.Bass("TRN2", target_bir_lowering=False, debug=debug, num_devices=8)

    mpA = nc.dram_tensor("mpA", [C_IN, MAW], BF16, kind="ExternalInput").ap()
    mpB = nc.dram_tensor("mpB", [4, MBW], BF16, kind="ExternalInput").ap()
    outp = nc.dram_tensor("outp", [C_OUT, 2 * N], F32, kind="ExternalOutput").ap()

    with tile.TileContext(nc) as tc, ExitStack() as ctx:
        consts = ctx.enter_context(tc.tile_pool(name="consts", bufs=1))
        work = ctx.enter_context(tc.tile_pool(name="work", bufs=1))
        tpool = ctx.enter_context(tc.tile_pool(name="tpool", bufs=NP))
        psum = ctx.enter_context(tc.tile_pool(name="psum", bufs=1, space="PSUM"))

        # ---- input loads, both on the SP HWDGE queue. B (the U path)
        # first: the whole T pipeline hangs off it, while the G path
        # tolerates mpA's later semaphore ----
        mpB_sb = consts.tile([4, MBW], BF16, tag="mpB")
        nc.sync.dma_start(out=mpB_sb, in_=mpB)
        mpA_sb = consts.tile([C_IN, MAW], BF16, tag="mpA")
        nc.sync.dma_start(out=mpA_sb, in_=mpA)

        fTq = mpA_sb[:, 0:64]
        gT1 = mpB_sb[:, 0:256]
        gTb1 = mpB_sb[:, 256:320]
        w1b = mpB_sb[:, 320:384]
        w1neg = mpB_sb[:, 384:448]

        # ---- PSUM tiles (8 banks of 2KB/partition) ----
        uneg_ps = psum.tile([2 * H, N], F32)    # -U^T on both partition halves
        ubT_ps = psum.tile([H, BQ], F32)
        warm_ps = psum.tile([C_OUT, 128], F32)  # warmers + observers target
        g2A = psum.tile([2 * H, 512], F32)      # G psum, pairs 0:16  (i, p')
        g2B = psum.tile([2 * H, 512], F32)      # G psum, pairs 16:32
        accA = psum.tile([C_OUT, N], F32)
        accB = psum.tile([C_OUT, N], F32)

        # ---- U matmuls: uneg = (-W1)^T @ g^T twice (both partition halves);
        # ubT = W1b^T @ gTb1 (b1 folded in via the ones row) ----
        nc.tensor.matmul(uneg_ps[0:H, :], lhsT=w1neg[0:3, :], rhs=gT1[0:3, :],
                         start=True, stop=True)
        nc.tensor.matmul(uneg_ps[H:2 * H, :], lhsT=w1neg[0:3, :],
                         rhs=gT1[0:3, :], start=True, stop=True)
        nc.tensor.matmul(ubT_ps, lhsT=w1b, rhs=gTb1, start=True, stop=True)

        # ---- shared T inputs ----
        # negua2[(bl,h), a] = -U[a, h] (bf16 so T ops hit fast DVE modes)
        negua2 = work.tile([2 * H, N], BF16, tag="negua2")
        nc.vector.tensor_copy(negua2, uneg_ps)
        # ubB2[(bl,h), p] = U[q0+bl*32+p, h] + b1[h]  (psum -> sbuf copies;
        # on DVE like negua2, so every T op needs only ONE cross-engine
        # wait (a single DVE counter covers both inputs — the Act/Pool
        # sequencers reject instructions with too many sync waits)
        ubB2 = work.tile([2 * H, NP], F32, tag="ubB2")
        nc.vector.tensor_copy(ubB2[0:H, :], ubT_ps[:, 0:32])
        nc.vector.tensor_copy(ubB2[H:2 * H, :], ubT_ps[:, 32:64])

        # ---- G matmuls, pair-half major so copies/chain overlap phase 2:
        # g2X[bl*64+h, i*16+p'] = G[q0+bl*32+(ph*16+p'), h, i] ----
        for ph, g2X in ((0, g2A), (1, g2B)):
            for bl in range(2):
                rhs = fTq[:, bl * 32 + ph * 16: bl * 32 + ph * 16 + 16]
                for i in range(C_OUT):
                    nc.tensor.matmul(
                        g2X[bl * H:(bl + 1) * H, i * 16:(i + 1) * 16],
                        lhsT=mpA_sb[:, 64 + i * 64:64 + (i + 1) * 64],
                        rhs=rhs, start=True, stop=True)

        # ---- G psum -> sbuf regroup: plain contiguous [128, 512] copies
        # (walrus rejects transposed copy APs); the (i, p') -> per-pair
        # transpose happens in the chain's lhsT access pattern instead.
        # g_sbuf[x][(bl,h), i*16 + p'] = g2X[(bl,h), i*16 + p']
        g_sbuf = [work.tile([2 * H, 512], BF16, name=f"gsb{x}",
                            tag=f"gsb{x}")
                  for x in range(2)]

        def g_copy(engine, x):
            g2X = (g2A, g2B)[x]
            if engine is nc.scalar:
                engine.activation(g_sbuf[x], g2X,
                                  mybir.ActivationFunctionType.Copy)
            else:
                engine.tensor_copy(g_sbuf[x], g2X)

        # lhsT view: [:, p, :] = [(bl,h), i] for pair p (free stride 16)
        g_lhs = [t.rearrange("k (i p) -> k p i", i=C_OUT) for t in g_sbuf]

        # copy-A gates the chain start: high priority on DVE (it pops the
        # moment G-A's semaphore lands). copy-B (Act) is emitted mid
        # T stream — it is only needed at pair 16.
        with tc.high_priority():
            g_copy(nc.vector, 0)

        # ---- T tiles ----
        t_tiles = [None] * NP

        def emit_t(p):
            t_p = tpool.tile([2 * H, N], BF16, tag="T", name=f"t_{p}")
            t_tiles[p] = t_p
            eng = T_ENGINES[p]
            if eng == "s":
                nc.scalar.activation(t_p, negua2,
                                     mybir.ActivationFunctionType.Relu,
                                     bias=ubB2[:, p:p + 1], scale=1.0)
            else:
                e = nc.vector if eng == "v" else nc.gpsimd
                with tc.tile_wait_until(5.2e-3, enable=(eng == "v")):
                    # Floor DVE T builds past copy-A's readiness in the
                    # scheduler's internal clock (which misses the fast DVE
                    # modes and would otherwise pack DVE before copy-A).
                    e.tensor_scalar(out=t_p, in0=negua2,
                                    scalar1=ubB2[:, p:p + 1], scalar2=0.0,
                                    op0=mybir.AluOpType.add,
                                    op1=mybir.AluOpType.max)

        for p in range(7):
            emit_t(p)
        g_copy(nc.scalar, 1)
        for p in range(7, NP):
            emit_t(p)

        # ---- observers: PE absorbs the g_sbuf copy semaphores so chain
        # matmuls carry only their T-tile wait (walrus: 1 wait/Matmult) ----
        def observe(x):
            nc.tensor.matmul(warm_ps[0:1, 0:1],
                             lhsT=g_sbuf[x][0:64, 0:1],
                             rhs=g_sbuf[x][0:64, 0:1],
                             start=True, stop=True)

        # ---- main chain: two pair-halves of acc, host sums them ----
        for p in range(NP):
            if p % 16 == 0:
                observe(p // 16)
            accX = accA if p < 16 else accB
            nc.tensor.matmul(accX, lhsT=g_lhs[p // 16][:, p % 16, :],
                             rhs=t_tiles[p],
                             start=(p % 16 == 0), stop=(p % 16 == 15))

        # ---- store: half A as soon as its group stops, half B at the end.
        # Both copies on Act (idle in this window) so DVE's T stream is
        # never interrupted ----
        out_sbA = work.tile([C_OUT, N], F32, tag="outA")
        out_sbB = work.tile([C_OUT, N], F32, tag="outB")
        with tc.high_priority():
            nc.scalar.activation(out_sbA, accA,
                                 mybir.ActivationFunctionType.Copy)
        nc.sync.dma_start(out=outp[:, 0:N], in_=out_sbA)
        with tc.high_priority():
            nc.scalar.activation(out_sbB, accB,
                                 mybir.ActivationFunctionType.Copy)
        nc.sync.dma_start(out=outp[:, N:2 * N], in_=out_sbB)

        # ---- PE warm-up / gap fillers: emitted last => lowest scheduler
        # priority => they run only when nothing else is ready on PE,
        # keeping the p-state ramp alive with zero semaphore waits ----
        wt_t = nc.alloc_sbuf_tensor("warm_src", [128, 128], BF16)
        wt = wt_t.ap()
        for _ in range(N_WARM):
            nc.tensor.matmul(warm_ps, lhsT=wt[:, 0:C_OUT], rhs=wt,
                             start=True, stop=True)

    # The framework registers four const-AP tensors ([128,1] fills) at Bass
    # init; nothing in this kernel reads them (the BIR verifier itself flags
    # them as reader-less), but their Pool memsets delay the preamble
    # all-engine barrier by ~450ns. Drop them.
    b0 = nc.m.functions[0].blocks[0]
    b0.instructions = [i for i in b0.instructions
                       if type(i).__name__ != "InstMemset"]
    return nc


def shard_inputs(features, geometry, W1, b1, W2, b2) -> list[dict]:
    import ml_dtypes
    bf16 = ml_dtypes.bfloat16
    f = np.ascontiguousarray(np.asarray(features, np.float32))
    g = np.ascontiguousarray(np.asarray(geometry, np.float32))
    W1 = np.ascontiguousarray(np.asarray(W1, np.float32))
    b1 = np.ascontiguousarray(np.asarray(b1, np.float32))
    W2 = np.ascontiguousarray(np.asarray(W2, np.float32))

    # M2v2[j, i*64+h] = W2[h, i*C_IN+j]
    m2v2 = W2.reshape(H, C_OUT, C_IN).transpose(2, 1, 0).reshape(C_IN, C_OUT * H)

    maps = []
    for core in range(8):
        z, q = divmod(core, 4)
        sl = slice(q * BQ, (q + 1) * BQ)
        mpA = np.zeros((C_IN, MAW), bf16)
        mpA[:, 0:64] = f[z, sl].T.astype(bf16)
        mpA[:, 64:2112] = m2v2.astype(bf16)
        mpB = np.zeros((4, MBW), bf16)
        mpB[0:3, 0:256] = g[z].T.astype(bf16)
        mpB[3, 0:256] = 1.0
        mpB[0:3, 256:320] = g[z, sl].T.astype(bf16)
        mpB[3, 256:320] = 1.0
        mpB[0:3, 320:384] = W1.astype(bf16)
        mpB[3, 320:384] = b1.astype(bf16)
        mpB[0:3, 384:448] = (-W1).astype(bf16)
        maps.append({"mpA": mpA, "mpB": mpB})
    return maps


def unshard(parts: list[np.ndarray], features, b2) -> np.ndarray:
    f = np.asarray(features, np.float32)
    b2m = np.asarray(b2, np.float32).reshape(C_OUT, C_IN)
    # bias[z, i] = sum_j b2[i, j] * (sum_b f[z, b, j]) — constant over a
    bias = f.sum(axis=1) @ b2m.T  # [Z, C_OUT]
    out = np.empty((Z, N, C_OUT), np.float32)
    for z in range(Z):
        acc = parts[4 * z][:, 0:N] + parts[4 * z][:, N:2 * N]
        for q in range(1, 4):
            acc = acc + parts[4 * z + q][:, 0:N] + parts[4 * z + q][:, N:2 * N]
        out[z] = acc.T + bias[z][None, :]
    return out


def kernel(**inputs) -> np.ndarray:
    nc = build_nc(debug=False)
    in_maps = shard_inputs(**inputs)
    res = run_bass_kernel_spmd(nc, in_maps, list(range(8)))
    return unshard([r["outp"] for r in res.results],
                   inputs["features"], inputs["b2"])
